# revision 54
# baseline (speedup 1.0000x reference)
"""Per-pixel dynamic 5x5 conv (KernelConv) on 8 Trainium2 NeuronCores.

out[b,c,h,w] = sum_{i,j} core[b,(i*5+j)*C+c,h,w] * pad(data)[b,c,h+i,w+j]

Sharding: channel groups of 8 per core (x 4 batches = 32 channel-images/core).
Layout on chip: partitions = h (128), free dim = image-blocks x w.

All HBM traffic is bf16 (host converts): halves the DMA bytes vs f32, and the
2e-2 rel-err budget dwarfs bf16 rounding (~0.5% worst case here).

The DMA roofline (360 GB/s shared across all queues in the cost model) is the
binding constraint, so the kernel minimizes bytes moved:
- core: 26.2 MB/core, read once as 100 per-k group slices.
- data: loaded ONCE (padded rows 0..127 full-width + a 4-row bottom tile).
  The other four row-shifted copies the taps need are built ON CHIP by the
  TensorEngine: shifted-identity matmuls (S_i = eye shifted by i, plus a
  4-row fixup from the bottom tile) into PSUM, ACT-copied back to bf16 SBUF
  512-col chunks at a time. This replaces ~4.3 MB of duplicate HBM reads.
  (Row shifts cannot be done with plain engine copies: SBUF access patterns
  may only start at partitions 0/32/64/96; only PE matmul or DMA cross
  partitions, and in the cost model DMA bandwidth is the bottleneck.)
- out: written as bf16, per 512-col PSUM bank.

Compute structure (all-bf16 operands keep the DVE 2x perf mode; a mixed f32
output would forfeit it):
- DVE does only the 25 per-k products per group.
- The 24 adds per pixel run on the TensorEngine as identity matmuls
  accumulating into PSUM (f32, exact): one matmul per 512-col bank per
  product. Taps run i-major so shift plane i+1 is PE-built (planes 1-2 while
  PE idles at the head, planes 3-4 at one chunk per group-0 tap) before
  round i+1 consumes it.
- ACT drains each finished PSUM bank to bf16 (the last group's second bank
  drains on the idle DVE instead, dodging ACT serialization); all out DMAs
  are issued on the sync ring after the last group's loads so their
  transfers fill the post-input DMA window instead of delaying the last
  input, with the final two on separate rings.
- Groups of (16, 8, 8) images: group 0 is wide so PE fits its accumulation
  plus all shift-plane builds inside its window; the small last group keeps
  the final mul->matmul->drain->out chain short. PSUM = 4 accum banks for
  group 0 + 4 shift-chunk banks that group 1 reuses for its accumulation
  (group 2 reuses group 0's). 728ns slice DMAs outpace the 565ns per-DMA
  ring issue rate, keeping the DMA device saturated.
"""

import numpy as np
import ml_dtypes

B, C, H, W = 4, 64, 128, 128
K, PAD, KK = 5, 2, 25
NCORES = 8
CPC = C // NCORES            # channels per core = 8
NIMG = B * CPC               # channel-images per core = 32
GRPS = (16, 8, 8)            # images per compute group: first is wide so its
                             # 7.3us tap rounds fit the plane builds; last is
                             # small for a short tail
NG = len(GRPS)
WP = W + 2 * PAD             # 132
HP = H + 2 * PAD             # 132
CHUNK = 512                  # PSUM bank = 512 f32 per partition
FW = NIMG * WP               # full padded width = 4224

BF16 = ml_dtypes.bfloat16
FP8 = ml_dtypes.float8_e3m4
NFP8 = 4                     # group-0 taps (i=0, j<NFP8) read core as fp8
FP8S = 16.0                  # prescale so 0.1-scale values clear e3m4's
                             # subnormal range; the PE accum matmul uses an
                             # eye/16 weight so compensation is exact

_CACHE = {}


def _shift_chunks():
    out = []
    off = 0
    while off < FW:
        n = min(CHUNK, FW - off)
        out.append((off, n))
        off += n
    return out


def _build_module(debug=False):
    import concourse.tile as tile
    from concourse import bacc, bass, mybir

    f32 = mybir.dt.float32
    bf16 = mybir.dt.bfloat16
    nc = bacc.Bacc(
        "TRN2", target_bir_lowering=False, debug=debug, num_devices=NCORES
    )
    core_d = nc.dram_tensor(
        "core", [KK, H, NIMG * W], bf16, kind="ExternalInput"
    ).ap()
    data_d = nc.dram_tensor(
        "data", [HP, FW], bf16, kind="ExternalInput"
    ).ap()
    ident_d = nc.dram_tensor(
        "ident", [128, 2 * 128], bf16, kind="ExternalInput"
    ).ap()
    core8_d = nc.dram_tensor(
        "core8", [NFP8, H, GRPS[0] * W], mybir.dt.float8e3,
        kind="ExternalInput",
    ).ap()
    smat_d = nc.dram_tensor(
        "smat", [128, (K - 1) * 128], bf16, kind="ExternalInput"
    ).ap()
    fmat_d = nc.dram_tensor(
        "fmat", [2 * PAD, (K - 1) * 128], bf16, kind="ExternalInput"
    ).ap()
    out_d = nc.dram_tensor("out", [H, NIMG * W], bf16, kind="ExternalOutput").ap()

    chunks = _shift_chunks()

    with tile.TileContext(nc) as tc:
        with (
            tc.tile_pool(name="constp", bufs=1) as k_pool,
            tc.tile_pool(name="shifts", bufs=1) as sh_pool,
            tc.tile_pool(name="corep", bufs=9) as ca_pool,
            tc.tile_pool(name="core8p", bufs=4) as c8_pool,
            tc.tile_pool(name="corepb", bufs=16) as cb_pool,
            tc.tile_pool(name="prodp", bufs=8) as pa_pool,
            tc.tile_pool(name="prodpb", bufs=8) as pb_pool,
            tc.tile_pool(name="outp", bufs=NG * 2) as o_pool,
            tc.tile_pool(
                name="psump", bufs=4, space=bass.MemorySpace.PSUM
            ) as ps_pool,
            tc.tile_pool(
                name="pshp", bufs=4, space=bass.MemorySpace.PSUM
            ) as psh_pool,
        ):
            identw = k_pool.tile([128, 2 * 128], bf16, tag="identw")
            nc.scalar.dma_start(identw[:], ident_d[:, :])
            ident = identw[:, :128]       # eye
            sident = identw[:, 128:]      # eye / FP8S
            smat = k_pool.tile([128, (K - 1) * 128], bf16, tag="smat")
            nc.scalar.dma_start(smat[:], smat_d[:, :])
            fmat = k_pool.tile([2 * PAD, (K - 1) * 128], bf16, tag="fmat")
            nc.scalar.dma_start(fmat[:], fmat_d[:, :])
            dpb = k_pool.tile([2 * PAD, FW], bf16, tag="dpb")
            nc.scalar.dma_start(dpb[:], data_d[H:HP, :])
            # full-width shift planes, one tile per row shift i so a tap's
            # product waits only on its own plane's writers
            shp = []
            for i in range(K):
                t = sh_pool.tile([H, FW], bf16, tag=f"shp{i}", name=f"shp{i}")
                shp.append(t)
            nc.scalar.dma_start(shp[0][:], data_d[0:H, :])
            nchunks = len(chunks)

            def emit_shift_chunk(i, ci):
                off, n = chunks[ci]
                psh = psh_pool.tile(
                    [H, CHUNK], f32, tag="psh", name=f"psh{i}_{ci}"
                )
                nc.tensor.matmul(
                    psh[:, :n],
                    smat[:, (i - 1) * 128 : i * 128],
                    shp[0][:, off : off + n],
                    start=True,
                    stop=False,
                )
                nc.tensor.matmul(
                    psh[:, :n],
                    fmat[:, (i - 1) * 128 : i * 128],
                    dpb[:, off : off + n],
                    start=False,
                    stop=True,
                )
                nc.scalar.copy(shp[i][:, off : off + n], psh[:, :n])

            # head start: planes 1-2 fully built while PE is otherwise
            # idle waiting for the first products; planes 3/4 follow at one
            # chunk per tap (a chunk pair + 4 accum matmuls fits in a
            # 1.46us tap slot; two pairs would overcommit PE ~0.25us/tap)
            for ci in range(nchunks):
                emit_shift_chunk(1, ci)
            for ci in range(nchunks):
                emit_shift_chunk(2, ci)
            outs_pending = []
            for g, grp in enumerate(GRPS):
                g0 = sum(GRPS[:g])
                gw_cols = grp * W
                nch = gw_cols // CHUNK
                gw = slice(g0 * W, g0 * W + gw_cols)
                last_g = g == NG - 1
                # g1's accum banks come from the shift-chunk pool (dead
                # after plane building) so g1 never waits on g0's drains
                apool = psh_pool if g == 1 else ps_pool
                pss = [
                    apool.tile([H, CHUNK], f32, tag="ps" if g != 1 else "psh",
                               name=f"ps{g}_{c}")
                    for c in range(nch)
                ]
                bpc = CHUNK // W  # images per 512-col bank = 4
                started = False
                for i in range(K):
                    # group 0 round 0 leads with its big bf16 tap so the
                    # DMA issue-pipeline warmup hides under a 1.8us transfer
                    # instead of starving the short fp8 transfers
                    js = [K - 1, 0, 1, 2, 3] if (g == 0 and i == 0) else range(K)
                    for j in js:
                        stop = i == K - 1 and j == K - 1
                        fp8_tap = g == 0 and i == 0 and j < NFP8
                        if last_g and i == K - 1 and j >= K - 2:
                            # last two taps of the final group run per-bank:
                            # the post-DMA chain (mul -> stop-matmul ->
                            # drain -> out) starts ~1us earlier for the
                            # closing bank than with full-width muls
                            for c in range(nch):
                                gwc = slice(
                                    g0 * W + c * CHUNK,
                                    g0 * W + (c + 1) * CHUNK,
                                )
                                cbc = cb_pool.tile(
                                    [H, CHUNK], bf16, tag="ctb",
                                    name=f"ctb_{g}_{i}_{j}_{c}",
                                )
                                nc.sync.dma_start(
                                    cbc[:], core_d[i * K + j, :, gwc]
                                )
                                pbc = pb_pool.tile(
                                    [H, CHUNK], bf16, tag="ptb",
                                    name=f"ptb_{g}_{i}_{j}_{c}",
                                )
                                shv = shp[i].rearrange(
                                    "p (b w) -> p b w", b=NIMG
                                )
                                bs = slice(
                                    g0 + c * bpc, g0 + (c + 1) * bpc
                                )
                                nc.vector.tensor_mul(
                                    pbc.rearrange("p (b w) -> p b w", b=bpc),
                                    cbc.rearrange("p (b w) -> p b w", b=bpc),
                                    shv[:, bs, j : j + W],
                                )
                                nc.tensor.matmul(
                                    pss[c][:], ident, pbc[:],
                                    start=False, stop=stop,
                                )
                            continue
                        if fp8_tap:
                            cti = c8_pool.tile(
                                [H, gw_cols], mybir.dt.float8e3, tag="ct8",
                                name=f"ct8_{j}",
                            )
                            nc.sync.dma_start(cti[:], core8_d[j, :, :])
                        else:
                            cti = (ca_pool if grp == 16 else cb_pool).tile(
                                [H, gw_cols], bf16, tag=f"ct{grp}",
                                name=f"ct_{g}_{i}_{j}",
                            )
                            nc.sync.dma_start(
                                cti[:], core_d[i * K + j, :, gw]
                            )
                        pti = (pa_pool if grp == 16 else pb_pool).tile(
                            [H, gw_cols], bf16, tag=f"pt{grp}",
                            name=f"pt_{g}_{i}_{j}",
                        )
                        shv = shp[i].rearrange("p (b w) -> p b w", b=NIMG)
                        nc.vector.tensor_mul(
                            pti.rearrange("p (b w) -> p b w", b=grp),
                            cti.rearrange("p (b w) -> p b w", b=grp),
                            shv[:, g0 : g0 + grp, j : j + W],
                        )
                        for c in range(nch):
                            nc.tensor.matmul(
                                pss[c][:],
                                sident if fp8_tap else ident,
                                pti[:, c * CHUNK : (c + 1) * CHUNK],
                                start=not started,
                                stop=stop,
                            )
                        started = True
                        if g == 0:
                            t = i * K + j  # tap index 0..24
                            if t < nchunks:
                                emit_shift_chunk(3, t)
                            elif t < 2 * nchunks:
                                emit_shift_chunk(4, t - nchunks)
                for c in range(nch):
                    gwc = slice(
                        g0 * W + c * CHUNK, g0 * W + (c + 1) * CHUNK
                    )
                    otc = o_pool.tile(
                        [H, CHUNK], bf16, tag="ot", name=f"ot{g}_{c}"
                    )
                    if last_g and c % 2 == 1:
                        # split the final drains across ACT and the
                        # now-idle DVE so they don't serialize
                        nc.vector.tensor_copy(otc[:], pss[c][:])
                    else:
                        nc.scalar.copy(otc[:], pss[c][:])
                    outs_pending.append((gwc, otc))
                if last_g:
                    # hidden groups' out DMAs issue on the sync ring here,
                    # after the last group's loads: issued earlier they would
                    # interleave into the input stream and push the last
                    # input (and the whole tail chain) later, while the
                    # post-input DMA window sat idle. The last group's two
                    # outs ride separate rings to dodge issue serialization.
                    for gwc, otc in outs_pending[:-2]:
                        nc.sync.dma_start(out_d[:, gwc], otc[:])
                    nc.scalar.dma_start(*(
                        (out_d[:, outs_pending[-2][0]], outs_pending[-2][1][:])
                    ))
                    nc.sync.dma_start(out_d[:, outs_pending[-1][0]],
                                      outs_pending[-1][1][:])

    nc.compile()
    return nc


def get_nc(debug=False):
    key = ("nc", debug)
    if key not in _CACHE:
        _CACHE[key] = _build_module(debug=debug)
    return _CACHE[key]


def prep_inputs(data, core):
    """Full inputs -> list of per-core input dicts (host-side shard + pad)."""
    data = np.asarray(data, dtype=np.float32)
    core = np.asarray(core, dtype=np.float32)
    # [b, i, j, c, h, w] -> [i, j, h, b, c, w]: k-slices i-major on device
    core_t = np.ascontiguousarray(
        core.reshape(B, K, K, C, H, W).transpose(1, 2, 4, 0, 3, 5).astype(BF16)
    )
    dp = np.zeros((HP, B, C, WP), BF16)
    dp[PAD : PAD + H, :, :, PAD : PAD + W] = data.transpose(2, 0, 1, 3)
    ident = np.zeros((128, 2 * 128), BF16)
    ident[:, :128] = np.eye(128, dtype=BF16)
    ident[:, 128:] = np.eye(128, dtype=BF16) * np.float32(1.0 / FP8S)
    # S_i shifts rows up by i via lhsT.T @ x: S_i[p, m] = 1 iff p == m + i;
    # F_i patches rows m >= 128 - i from the 4-row bottom tile
    smat = np.zeros((128, (K - 1) * 128), BF16)
    fmat = np.zeros((2 * PAD, (K - 1) * 128), BF16)
    for i in range(1, K):
        smat[:, (i - 1) * 128 : i * 128] = np.eye(128, k=-i, dtype=BF16)
        for p in range(2 * PAD):
            m = 128 + p - i
            if 0 <= m < 128:
                fmat[p, (i - 1) * 128 + m] = 1.0
    in_maps = []
    for r in range(NCORES):
        cs = slice(r * CPC, (r + 1) * CPC)
        core_r = np.ascontiguousarray(core_t[:, :, :, :, cs, :]).reshape(
            KK, H, NIMG * W
        )
        # group-0 columns of taps (i=0, j<NFP8), prescaled into e3m4
        core8_r = np.ascontiguousarray(
            (core_r[:NFP8, :, : GRPS[0] * W].astype(np.float32) * FP8S)
            .astype(FP8)
        )
        data_r = np.ascontiguousarray(dp[:, :, cs, :]).reshape(HP, NIMG * WP)
        in_maps.append(
            {
                "core": core_r,
                "core8": core8_r,
                "data": data_r,
                "ident": ident,
                "smat": smat,
                "fmat": fmat,
            }
        )
    return in_maps


def assemble(per_core_outs):
    """Per-core 'out' arrays [H, NIMG*W] (bf16) -> full [B, C, H, W] f32."""
    out = np.empty((B, C, H, W), np.float32)
    for r, o in enumerate(per_core_outs):
        cs = slice(r * CPC, (r + 1) * CPC)
        out[:, cs] = (
            np.asarray(o).astype(np.float32)
            .reshape(H, B, CPC, W)
            .transpose(1, 2, 0, 3)
        )
    return out


def run_spmd(in_maps, trace=False, trace_cores=None):
    from concourse.bass_utils import run_bass_kernel_spmd

    return run_bass_kernel_spmd(
        get_nc(),
        in_maps,
        list(range(NCORES)),
        trace=trace,
        trace_cores=trace_cores,
    )


def kernel(data, core):
    res = run_spmd(prep_inputs(data, core))
    return assemble([res.results[r]["out"] for r in range(NCORES)])
